# revision 16
# baseline (speedup 1.0000x reference)
"""CPSF codebook fused kernel for 8 Trainium2 NeuronCores.

Math (see reference): for each batch row b and codebook entry m,
  q[b,m] = par_sq/s_par + (max(tot_sq-par_sq,0) + max(dd_sq,0))/s_perp
  w[b,m] = alpha[m] * exp(-pi*q)
  out    = Re((w @ (T_hat_re + i*T_hat_im)) @ A.T),  A = exp(i*2pi/S * k*s)

Device strategy (pure batch-parallel, no collectives):
  - Since w is real, the final DFT is folded into the codebook on the host:
      out = w @ TA,   TA = T_hat_re @ cos(ang) - T_hat_im @ sin(ang)
  - Everything per-(b,m) is expressed as fp32r matmuls against host-packed
    operand stacks plus a tiny per-element epilogue (2 ACT squares, 2 DVE
    ops, 1 ACT exp). The relu clamps are dropped: tot_sq-par_sq >= 0 and
    dd_sq >= 0 hold mathematically (Cauchy-Schwarz / norms); clamping only
    trims float roundoff of ~1e-6 which is far below output noise.
  - Each core handles B/8 = 512 batch rows against the full codebook,
    streaming the codebook operand stacks from DRAM.
"""

import os
import sys

for _p in ("/opt/trn_rl_repo", os.path.expanduser("~/.axon_site/_ro/trn_rl_repo")):
    if os.path.isdir(_p) and _p not in sys.path:
        sys.path.insert(0, _p)

import numpy as np

B, N, M, S = 4096, 64, 8192, 256
NCORES = 8
BLOC = B // NCORES          # 512 batch rows per core
NT = M // 128               # 64 codebook tiles
PI = float(np.pi)


def _prep(x_re, x_im, z_j_re, z_j_im, vec_d_j_re, vec_d_j_im,
          T_hat_re, T_hat_im, alpha_j, sigma_par, sigma_perp):
    """Host-side operand packing (all O(B*N + M*N + M*S^2) — tiny vs device work)."""
    f32 = np.float32
    tiny = np.finfo(f32).tiny

    # ---- batch side ----
    z_re = np.ascontiguousarray(x_re[:, :N]).astype(f32)
    z_im = np.ascontiguousarray(x_im[:, :N]).astype(f32)
    vd_re = np.ascontiguousarray(x_re[:, N:]).astype(f32)
    vd_im = np.ascontiguousarray(x_im[:, N:]).astype(f32)
    nrm = np.sqrt((vd_re * vd_re + vd_im * vd_im).sum(-1, dtype=f32)).astype(f32)
    nrm = np.where(nrm == 0, f32(1.0), nrm)
    vd_re = vd_re / nrm[:, None]
    vd_im = vd_im / nrm[:, None]
    z_sq = (z_re * z_re + z_im * z_im).sum(-1, dtype=f32)
    vd_sq = (vd_re * vd_re + vd_im * vd_im).sum(-1, dtype=f32)

    r1 = np.ascontiguousarray(np.concatenate([z_re.T, z_im.T], 0))      # [128, B]
    r2 = np.ascontiguousarray(np.concatenate([vd_re.T, vd_im.T], 0))    # [128, B]
    vrow = (z_sq + vd_sq).astype(f32)[None, :]                          # [1, B]

    # ---- codebook side ----
    djr = z_j_re.astype(f32)  # names: z_j
    dji = z_j_im.astype(f32)
    vr = vec_d_j_re.astype(f32)
    vi = vec_d_j_im.astype(f32)
    nj = np.sqrt((vr * vr + vi * vi).sum(-1, dtype=f32)).astype(f32)
    nj = np.where(nj == 0, f32(1.0), nj)
    vr = vr / nj[:, None]
    vi = vi / nj[:, None]

    alpha = np.maximum(alpha_j.astype(f32), tiny)
    s_par = np.maximum(sigma_par.astype(f32), tiny)
    s_perp = np.maximum(sigma_perp.astype(f32), tiny)
    sp12 = (1.0 / np.sqrt(s_perp.astype(np.float64))).astype(f32)       # s_perp^-1/2
    inv_sp = (1.0 / s_perp.astype(np.float64)).astype(f32)

    # c0 = <conj(vd_j), z_j> per m
    c0_re = (vr * djr + vi * dji).sum(-1, dtype=f32)
    c0_im = (vr * dji - vi * djr).sum(-1, dtype=f32)
    z_j_sq = (djr * djr + dji * dji).sum(-1, dtype=f32)
    vd_j_sq = (vr * vr + vi * vi).sum(-1, dtype=f32)

    # lhsT stacks [K=128, M], scaled per column m
    L1 = np.concatenate([vr.T, vi.T], 0) * sp12[None, :]
    L2 = np.concatenate([-vi.T, vr.T], 0) * sp12[None, :]
    L3 = np.concatenate([djr.T, dji.T], 0) * (-2.0 * inv_sp)[None, :]
    L4 = np.concatenate([vr.T, vi.T], 0) * (-2.0 * inv_sp)[None, :]

    # per-m epilogue scalars, tile-packed: mv[p, t*4+j]
    b1 = -(c0_re * sp12)                       # ACT square bias for G1
    b2 = -(c0_im * sp12)                       # G2 shift
    cst = (z_j_sq + vd_j_sq) * inv_sp
    rm1_64 = s_perp.astype(np.float64) / s_par.astype(np.float64) - 1.0
    rm1 = rm1_64.astype(f32)
    be = (np.log(alpha.astype(np.float64))
          - PI * cst.astype(np.float64)).astype(f32)
    mv = np.empty((128, NT * 4), f32)
    for t in range(NT):
        sl = slice(t * 128, (t + 1) * 128)
        mv[:, t * 4 + 0] = b1[sl]
        mv[:, t * 4 + 1] = b2[sl]
        mv[:, t * 4 + 2] = be[sl]
        mv[:, t * 4 + 3] = rm1[sl]

    # DFT folded into the codebook: TA = T_hat_re @ cos - T_hat_im @ sin.
    # Angles replicate the reference's fp32 computation exactly.
    nn = np.arange(S, dtype=f32)
    ang = f32(2.0 * np.pi / S) * (nn[:, None] * nn[None, :])
    cosA = np.cos(ang).astype(f32)
    sinA = np.sin(ang).astype(f32)
    TA = (T_hat_re.astype(np.float64) @ cosA.astype(np.float64)
          - T_hat_im.astype(np.float64) @ sinA.astype(np.float64)).astype(f32)
    TA = TA.reshape(NT, 128, S)

    # The v[b]-term of q is inv_sp[m]*v[b]. Split inv_sp = c0 + delta:
    # the c0*v[b] part factors out of the exp as a per-b output row scale
    # (exact); only a nonzero delta needs the rank-1 matmul on device.
    c0v = float(inv_sp.mean(dtype=np.float64))
    delta = (inv_sp.astype(np.float64) - c0v).astype(f32)
    uniform = bool(np.all(delta == 0))
    vraw = (z_sq + vd_sq).astype(np.float64)
    erow = np.exp(-PI * c0v * vraw).astype(f32)          # [B] output row scale
    osc = np.ascontiguousarray(erow.reshape(NCORES, 4, 128).transpose(0, 2, 1))

    # pack per m-tile: [NT, 128, 768] with cols [L1|L2|L3|L4|TA]
    lpack = np.empty((NT, 128, 768), f32)
    for t in range(NT):
        sl = slice(t * 128, (t + 1) * 128)
        lpack[t, :, 0:128] = L1[:, sl]
        lpack[t, :, 128:256] = L2[:, sl]
        lpack[t, :, 256:384] = L3[:, sl]
        lpack[t, :, 384:512] = L4[:, sl]
        lpack[t, :, 512:768] = TA[t]

    return dict(r1=r1, r2=r2, vrow=vrow, lpack=lpack, mv=mv,
                inv_sp=np.ascontiguousarray(delta[None, :]), osc=osc,
                uniform=uniform)


_CACHED = {}


def _build_nc(uniform):
    key = ("nc", uniform)
    if key in _CACHED:
        return _CACHED[key]
    import concourse.bacc as bacc
    import concourse.masks as masks
    import concourse.mybir as mybir
    import concourse.tile as tile

    F32 = mybir.dt.float32
    F32R = mybir.dt.float32r
    AF = mybir.ActivationFunctionType
    OP = mybir.AluOpType

    nc = bacc.Bacc("TRN2", target_bir_lowering=False, debug=False,
                   num_devices=NCORES)
    d_r1 = nc.dram_tensor("r1", [128, BLOC], F32R, kind="ExternalInput").ap()
    d_r2 = nc.dram_tensor("r2", [128, BLOC], F32R, kind="ExternalInput").ap()
    d_v = nc.dram_tensor("vrow", [1, BLOC], F32R, kind="ExternalInput").ap()
    d_lp = nc.dram_tensor("lpack", [NT, 128, 768], F32R, kind="ExternalInput").ap()
    d_mv = nc.dram_tensor("mv", [128, NT * 4], F32, kind="ExternalInput").ap()
    d_isp = nc.dram_tensor("inv_sp", [1, M], F32R, kind="ExternalInput").ap()
    d_osc = nc.dram_tensor("osc", [128, 4], F32, kind="ExternalInput").ap()
    d_out = nc.dram_tensor("out", [BLOC, S], F32, kind="ExternalOutput").ap()

    with tile.TileContext(nc) as tc:
        with tc.tile_pool(name="const", bufs=1) as cp, \
             tc.tile_pool(name="lp", bufs=12) as lpool, \
             tc.tile_pool(name="g", bufs=2, space="PSUM") as gpool, \
             tc.tile_pool(name="tacc", bufs=1, space="PSUM") as taccp, \
             tc.tile_pool(name="u", bufs=3) as upool, \
             tc.tile_pool(name="w", bufs=3) as wpool:
            r1 = cp.tile([128, BLOC], F32R)
            r2 = cp.tile([128, BLOC], F32R)
            vrow = cp.tile([1, BLOC], F32R)
            mv = cp.tile([128, NT * 4], F32)
            osc = cp.tile([128, 4], F32)
            ident = cp.tile([128, 128], F32)
            nc.sync.dma_start(r1[:], d_r1)
            nc.sync.dma_start(r2[:], d_r2)
            nc.sync.dma_start(vrow[:], d_v)
            nc.sync.dma_start(mv[:], d_mv)
            nc.sync.dma_start(osc[:], d_osc)
            masks.make_identity(nc, ident[:])
            if not uniform:
                isp = cp.tile([1, M], F32R)
                nc.sync.dma_start(isp[:], d_isp)

            # out.T accumulators: [128 k-half, 512 b]
            ot0 = taccp.tile([128, BLOC], F32)
            ot1 = taccp.tile([128, BLOC], F32)

            for t in range(NT):
                lp = lpool.tile([128, 768], F32R)
                nc.sync.dma_start(lp[:], d_lp[t])

                g1 = gpool.tile([128, BLOC], F32, tag="g1")
                g2 = gpool.tile([128, BLOC], F32, tag="g2")
                s = gpool.tile([128, BLOC], F32, tag="s")
                nc.tensor.matmul(g1[:], lp[:, 0:128], r1[:], start=True, stop=True)
                nc.tensor.matmul(g2[:], lp[:, 128:256], r1[:], start=True, stop=True)
                if uniform:
                    nc.tensor.matmul(s[:], lp[:, 256:384], r1[:],
                                     start=True, stop=False)
                    nc.tensor.matmul(s[:], lp[:, 384:512], r2[:],
                                     start=False, stop=True)
                else:
                    nc.tensor.matmul(s[:], lp[:, 256:384], r1[:],
                                     start=True, stop=False)
                    nc.tensor.matmul(s[:], lp[:, 384:512], r2[:],
                                     start=False, stop=False)
                    nc.tensor.matmul(s[:], isp[:, t * 128:(t + 1) * 128], vrow[:],
                                     start=False, stop=True)

                u1 = upool.tile([128, BLOC], F32, tag="u1")
                u2 = upool.tile([128, BLOC], F32, tag="u2")
                a2 = upool.tile([128, BLOC], F32, tag="a2")
                nc.scalar.activation(u1[:], g1[:], AF.Square,
                                     bias=mv[:, t * 4:t * 4 + 1], scale=1.0)
                nc.vector.tensor_scalar_add(a2[:], g2[:],
                                            mv[:, t * 4 + 1:t * 4 + 2])
                nc.vector.tensor_mul(u2[:], a2[:], a2[:])
                u = upool.tile([128, BLOC], F32, tag="u")
                nc.gpsimd.tensor_add(u[:], u1[:], u2[:])
                q = upool.tile([128, BLOC], F32, tag="q")
                nc.vector.scalar_tensor_tensor(
                    q[:], u[:], mv[:, t * 4 + 3:t * 4 + 4], s[:],
                    op0=OP.mult, op1=OP.add)
                w = wpool.tile([128, BLOC], F32R, tag="w")
                nc.scalar.activation(w[:], q[:], AF.Exp,
                                     bias=mv[:, t * 4 + 2:t * 4 + 3], scale=-PI)

                # out.T[k, b] += ta[m, k].T @ w[m, b]
                for h, oth in ((0, ot0), (1, ot1)):
                    nc.tensor.matmul(oth[:], lp[:, 512 + h * 128:512 + (h + 1) * 128],
                                     w[:], start=(t == 0), stop=(t == NT - 1),
                                     skip_group_check=True)

            # transpose out.T -> out, scaling rows by osc, then DMA out
            obs = [wpool.tile([128, S], F32, tag=f"ob{j}", bufs=1,
                              name=f"ob{j}") for j in range(4)]
            for h, oth in ((0, ot0), (1, ot1)):
                tsb = wpool.tile([128, BLOC], F32, tag="tsb")
                nc.scalar.copy(tsb[:], oth[:])
                for j in range(4):
                    pt = gpool.tile([128, 128], F32, tag="g1", name=f"pt{h}{j}")
                    nc.tensor.transpose(pt[:], tsb[:, j * 128:(j + 1) * 128],
                                        ident[:])
                    nc.vector.tensor_scalar_mul(
                        obs[j][:, h * 128:(h + 1) * 128], pt[:], osc[:, j:j + 1])
            for j in range(4):
                nc.sync.dma_start(d_out[j * 128:(j + 1) * 128, :], obs[j][:])
    nc.compile()
    _CACHED[key] = nc
    return nc


def _run(inputs, trace=False):
    from concourse.bass_utils import run_bass_kernel_spmd

    prep = _prep(**inputs)
    nc = _build_nc(prep["uniform"])
    shared = {k: prep[k] for k in ("lpack", "mv", "inv_sp")}
    in_maps = []
    for c in range(NCORES):
        sl = slice(c * BLOC, (c + 1) * BLOC)
        in_maps.append(dict(r1=np.ascontiguousarray(prep["r1"][:, sl]),
                            r2=np.ascontiguousarray(prep["r2"][:, sl]),
                            vrow=np.ascontiguousarray(prep["vrow"][:, sl]),
                            osc=np.ascontiguousarray(prep["osc"][c]),
                            **shared))
    res = run_bass_kernel_spmd(nc, in_maps, list(range(NCORES)), trace=trace)
    out = np.concatenate([res.results[c]["out"] for c in range(NCORES)], 0)
    return out.astype(np.float32), res


def kernel(**inputs):
    out, _ = _run(inputs, trace=False)
    return out


def _install_ntff_hook():
    """The agent image's antenv lacks axon_hooks; recreate it so trace=True
    can capture NTFF profiles via libaxon_pjrt.so (same mechanism as
    trn_agent_boot.trn_boot)."""
    import types

    try:
        from antenv.axon_hooks import get_axon_ntff_profile_hook  # noqa: F401
        return
    except ImportError:
        pass
    import contextlib
    import ctypes

    so_path = "/opt/axon/libaxon_pjrt.so"
    lib = ctypes.CDLL(so_path)
    lib.axon_start_nrt_profile.argtypes = [ctypes.POINTER(ctypes.c_int64),
                                           ctypes.c_size_t]
    lib.axon_start_nrt_profile.restype = ctypes.c_int64
    lib.axon_stop_nrt_profile.argtypes = [ctypes.c_char_p]
    lib.axon_stop_nrt_profile.restype = ctypes.c_int64

    @contextlib.contextmanager
    def _hook(output_dir, device_ids):
        import jax

        jax.devices()
        if device_ids:
            ids = (ctypes.c_int64 * len(device_ids))(*device_ids)
            rc = lib.axon_start_nrt_profile(ids, len(device_ids))
        else:
            rc = lib.axon_start_nrt_profile(None, 0)
        if rc != 0:
            raise RuntimeError(f"axon_start_nrt_profile rc={rc}")
        try:
            yield
        finally:
            n = lib.axon_stop_nrt_profile(str(output_dir).encode())
            if n < 0:
                raise RuntimeError(f"axon_stop_nrt_profile rc={n}")
            if n == 0:
                print("WARNING: NTFF capture wrote nothing (raced the execute)")

    mod = types.ModuleType("antenv.axon_hooks")
    mod.get_axon_ntff_profile_hook = lambda: _hook
    mod.set_axon_ntff_profile_hook = lambda h: None
    sys.modules["antenv.axon_hooks"] = mod
    import antenv

    antenv.axon_hooks = mod


def run_traced(inputs):
    _install_ntff_hook()
    return _run(inputs, trace=True)


# revision 18
# speedup vs baseline: 1.2951x; 1.2951x over previous
"""CPSF codebook fused kernel for 8 Trainium2 NeuronCores.

Math (see reference): for each batch row b and codebook entry m,
  q[b,m] = par_sq/s_par + (max(tot_sq-par_sq,0) + max(dd_sq,0))/s_perp
  w[b,m] = alpha[m] * exp(-pi*q)
  out    = Re((w @ (T_hat_re + i*T_hat_im)) @ A.T),  A = exp(i*2pi/S * k*s)

Device strategy (pure batch-parallel, no collectives):
  - Since w is real, the final DFT is folded into the codebook on the host:
      out = w @ TA,   TA = T_hat_re @ cos(ang) - T_hat_im @ sin(ang)
  - Everything per-(b,m) is expressed as fp32r matmuls against host-packed
    operand stacks plus a tiny per-element epilogue (2 ACT squares, 2 DVE
    ops, 1 ACT exp). The relu clamps are dropped: tot_sq-par_sq >= 0 and
    dd_sq >= 0 hold mathematically (Cauchy-Schwarz / norms); clamping only
    trims float roundoff of ~1e-6 which is far below output noise.
  - Each core handles B/8 = 512 batch rows against the full codebook,
    streaming the codebook operand stacks from DRAM.
"""

import os
import sys

for _p in ("/opt/trn_rl_repo", os.path.expanduser("~/.axon_site/_ro/trn_rl_repo")):
    if os.path.isdir(_p) and _p not in sys.path:
        sys.path.insert(0, _p)

import numpy as np

B, N, M, S = 4096, 64, 8192, 256
NCORES = 8
BLOC = B // NCORES          # 512 batch rows per core
NT = M // 128               # 64 codebook tiles
PI = float(np.pi)


def _prep(x_re, x_im, z_j_re, z_j_im, vec_d_j_re, vec_d_j_im,
          T_hat_re, T_hat_im, alpha_j, sigma_par, sigma_perp):
    """Host-side operand packing (all O(B*N + M*N + M*S^2) — tiny vs device work)."""
    f32 = np.float32
    tiny = np.finfo(f32).tiny

    # ---- batch side ----
    z_re = np.ascontiguousarray(x_re[:, :N]).astype(f32)
    z_im = np.ascontiguousarray(x_im[:, :N]).astype(f32)
    vd_re = np.ascontiguousarray(x_re[:, N:]).astype(f32)
    vd_im = np.ascontiguousarray(x_im[:, N:]).astype(f32)
    nrm = np.sqrt((vd_re * vd_re + vd_im * vd_im).sum(-1, dtype=f32)).astype(f32)
    nrm = np.where(nrm == 0, f32(1.0), nrm)
    vd_re = vd_re / nrm[:, None]
    vd_im = vd_im / nrm[:, None]
    z_sq = (z_re * z_re + z_im * z_im).sum(-1, dtype=f32)
    vd_sq = (vd_re * vd_re + vd_im * vd_im).sum(-1, dtype=f32)

    r1 = np.ascontiguousarray(np.concatenate([z_re.T, z_im.T], 0))      # [128, B]
    r2 = np.ascontiguousarray(np.concatenate([vd_re.T, vd_im.T], 0))    # [128, B]
    vrow = (z_sq + vd_sq).astype(f32)[None, :]                          # [1, B]

    # ---- codebook side ----
    djr = z_j_re.astype(f32)  # names: z_j
    dji = z_j_im.astype(f32)
    vr = vec_d_j_re.astype(f32)
    vi = vec_d_j_im.astype(f32)
    nj = np.sqrt((vr * vr + vi * vi).sum(-1, dtype=f32)).astype(f32)
    nj = np.where(nj == 0, f32(1.0), nj)
    vr = vr / nj[:, None]
    vi = vi / nj[:, None]

    alpha = np.maximum(alpha_j.astype(f32), tiny)
    s_par = np.maximum(sigma_par.astype(f32), tiny)
    s_perp = np.maximum(sigma_perp.astype(f32), tiny)
    sp12 = (1.0 / np.sqrt(s_perp.astype(np.float64))).astype(f32)       # s_perp^-1/2
    inv_sp = (1.0 / s_perp.astype(np.float64)).astype(f32)

    # c0 = <conj(vd_j), z_j> per m
    c0_re = (vr * djr + vi * dji).sum(-1, dtype=f32)
    c0_im = (vr * dji - vi * djr).sum(-1, dtype=f32)
    z_j_sq = (djr * djr + dji * dji).sum(-1, dtype=f32)
    vd_j_sq = (vr * vr + vi * vi).sum(-1, dtype=f32)

    # lhsT stacks [K=128, M], scaled per column m
    L1 = np.concatenate([vr.T, vi.T], 0) * sp12[None, :]
    L2 = np.concatenate([-vi.T, vr.T], 0) * sp12[None, :]
    L3 = np.concatenate([djr.T, dji.T], 0) * (-2.0 * inv_sp)[None, :]
    L4 = np.concatenate([vr.T, vi.T], 0) * (-2.0 * inv_sp)[None, :]

    # per-m epilogue scalars, tile-packed: mv[p, t*4+j]
    b1 = -(c0_re * sp12)                       # ACT square bias for G1
    b2 = -(c0_im * sp12)                       # G2 shift
    cst = (z_j_sq + vd_j_sq) * inv_sp
    rm1_64 = s_perp.astype(np.float64) / s_par.astype(np.float64) - 1.0
    rm1 = rm1_64.astype(f32)
    be = (np.log(alpha.astype(np.float64))
          - PI * cst.astype(np.float64)).astype(f32)
    mv = np.empty((128, NT * 4), f32)
    for t in range(NT):
        sl = slice(t * 128, (t + 1) * 128)
        mv[:, t * 4 + 0] = b1[sl]
        mv[:, t * 4 + 1] = b2[sl]
        mv[:, t * 4 + 2] = be[sl]
        mv[:, t * 4 + 3] = rm1[sl]

    # DFT folded into the codebook: TA = T_hat_re @ cos - T_hat_im @ sin.
    # Angles replicate the reference's fp32 computation exactly.
    nn = np.arange(S, dtype=f32)
    ang = f32(2.0 * np.pi / S) * (nn[:, None] * nn[None, :])
    cosA = np.cos(ang).astype(f32)
    sinA = np.sin(ang).astype(f32)
    TA = (T_hat_re.astype(np.float64) @ cosA.astype(np.float64)
          - T_hat_im.astype(np.float64) @ sinA.astype(np.float64)).astype(f32)
    TA = TA.reshape(NT, 128, S)

    # The v[b]-term of q is inv_sp[m]*v[b]. Split inv_sp = c0 + delta:
    # the c0*v[b] part factors out of the exp as a per-b output row scale
    # (exact); only a nonzero delta needs the rank-1 matmul on device.
    c0v = float(inv_sp.mean(dtype=np.float64))
    delta = (inv_sp.astype(np.float64) - c0v).astype(f32)
    uniform = bool(np.all(delta == 0))
    vraw = (z_sq + vd_sq).astype(np.float64)
    erow = np.exp(-PI * c0v * vraw).astype(f32)          # [B] output row scale
    osc = np.ascontiguousarray(erow.reshape(NCORES, 4, 128).transpose(0, 2, 1))

    # pack per m-tile: [NT, 128, 768] with cols [L1|L2|L3|L4|TA]
    lpack = np.empty((NT, 128, 768), f32)
    for t in range(NT):
        sl = slice(t * 128, (t + 1) * 128)
        lpack[t, :, 0:128] = L1[:, sl]
        lpack[t, :, 128:256] = L2[:, sl]
        lpack[t, :, 256:384] = L3[:, sl]
        lpack[t, :, 384:512] = L4[:, sl]
        lpack[t, :, 512:768] = TA[t]

    return dict(r1=r1, r2=r2, vrow=vrow, lpack=lpack, mv=mv,
                inv_sp=np.ascontiguousarray(delta[None, :]), osc=osc,
                uniform=uniform)


_CACHED = {}


def _build_nc(uniform):
    key = ("nc", uniform)
    if key in _CACHED:
        return _CACHED[key]
    import concourse.bacc as bacc
    import concourse.masks as masks
    import concourse.mybir as mybir
    import concourse.tile as tile

    F32 = mybir.dt.float32
    F32R = mybir.dt.float32r
    AF = mybir.ActivationFunctionType
    OP = mybir.AluOpType

    nc = bacc.Bacc("TRN2", target_bir_lowering=False, debug=False,
                   num_devices=NCORES)
    d_r1 = nc.dram_tensor("r1", [128, BLOC], F32R, kind="ExternalInput").ap()
    d_r2 = nc.dram_tensor("r2", [128, BLOC], F32R, kind="ExternalInput").ap()
    d_v = nc.dram_tensor("vrow", [1, BLOC], F32R, kind="ExternalInput").ap()
    d_lp = nc.dram_tensor("lpack", [NT, 128, 768], F32R, kind="ExternalInput").ap()
    d_mv = nc.dram_tensor("mv", [128, NT * 4], F32, kind="ExternalInput").ap()
    d_isp = nc.dram_tensor("inv_sp", [1, M], F32R, kind="ExternalInput").ap()
    d_osc = nc.dram_tensor("osc", [128, 4], F32, kind="ExternalInput").ap()
    d_out = nc.dram_tensor("out", [BLOC, S], F32, kind="ExternalOutput").ap()

    with tile.TileContext(nc) as tc:
        with tc.tile_pool(name="const", bufs=1) as cp, \
             tc.tile_pool(name="lp", bufs=12) as lpool, \
             tc.tile_pool(name="g", bufs=2, space="PSUM") as gpool, \
             tc.tile_pool(name="tacc", bufs=1, space="PSUM") as taccp, \
             tc.tile_pool(name="u", bufs=4) as upool, \
             tc.tile_pool(name="w", bufs=4) as wpool:
            r1 = cp.tile([128, BLOC], F32R)
            r2 = cp.tile([128, BLOC], F32R)
            vrow = cp.tile([1, BLOC], F32R)
            mv = cp.tile([128, NT * 4], F32)
            osc = cp.tile([128, 4], F32)
            ident = cp.tile([128, 128], F32)
            nc.sync.dma_start(r1[:], d_r1)
            nc.sync.dma_start(r2[:], d_r2)
            nc.sync.dma_start(vrow[:], d_v)
            nc.sync.dma_start(mv[:], d_mv)
            nc.sync.dma_start(osc[:], d_osc)
            masks.make_identity(nc, ident[:])
            if not uniform:
                isp = cp.tile([1, M], F32R)
                nc.sync.dma_start(isp[:], d_isp)

            # out.T accumulators: [128 k-half, 512 b]
            ot0 = taccp.tile([128, BLOC], F32)
            ot1 = taccp.tile([128, BLOC], F32)

            for t in range(NT):
                lp = lpool.tile([128, 768], F32R)
                nc.sync.dma_start(lp[:], d_lp[t])

                g1 = gpool.tile([128, BLOC], F32, tag="g1")
                g2 = gpool.tile([128, BLOC], F32, tag="g2")
                s = gpool.tile([128, BLOC], F32, tag="s")
                nc.tensor.matmul(g1[:], lp[:, 0:128], r1[:], start=True, stop=True)
                nc.tensor.matmul(g2[:], lp[:, 128:256], r1[:], start=True, stop=True)
                if uniform:
                    nc.tensor.matmul(s[:], lp[:, 256:384], r1[:],
                                     start=True, stop=False)
                    nc.tensor.matmul(s[:], lp[:, 384:512], r2[:],
                                     start=False, stop=True)
                else:
                    nc.tensor.matmul(s[:], lp[:, 256:384], r1[:],
                                     start=True, stop=False)
                    nc.tensor.matmul(s[:], lp[:, 384:512], r2[:],
                                     start=False, stop=False)
                    nc.tensor.matmul(s[:], isp[:, t * 128:(t + 1) * 128], vrow[:],
                                     start=False, stop=True)

                u1 = upool.tile([128, BLOC], F32, tag="u1")
                u2 = upool.tile([128, BLOC], F32, tag="u2")
                nc.scalar.activation(u1[:], g1[:], AF.Square,
                                     bias=mv[:, t * 4:t * 4 + 1], scale=1.0)
                nc.scalar.activation(u2[:], g2[:], AF.Square,
                                     bias=mv[:, t * 4 + 1:t * 4 + 2], scale=1.0)
                u = upool.tile([128, BLOC], F32, tag="u")
                nc.gpsimd.tensor_add(u[:], u1[:], u2[:])
                q = upool.tile([128, BLOC], F32, tag="q")
                nc.vector.scalar_tensor_tensor(
                    q[:], u[:], mv[:, t * 4 + 3:t * 4 + 4], s[:],
                    op0=OP.mult, op1=OP.add)
                w = wpool.tile([128, BLOC], F32R, tag="w")
                nc.scalar.activation(w[:], q[:], AF.Exp,
                                     bias=mv[:, t * 4 + 2:t * 4 + 3], scale=-PI)

                # out.T[k, b] += ta[m, k].T @ w[m, b]
                for h, oth in ((0, ot0), (1, ot1)):
                    nc.tensor.matmul(oth[:], lp[:, 512 + h * 128:512 + (h + 1) * 128],
                                     w[:], start=(t == 0), stop=(t == NT - 1),
                                     skip_group_check=True)

            # transpose out.T -> out, scaling rows by osc, then DMA out
            obs = [wpool.tile([128, S], F32, tag=f"ob{j}", bufs=1,
                              name=f"ob{j}") for j in range(4)]
            for h, oth in ((0, ot0), (1, ot1)):
                tsb = wpool.tile([128, BLOC], F32, tag="tsb")
                nc.scalar.copy(tsb[:], oth[:])
                for j in range(4):
                    pt = gpool.tile([128, 128], F32, tag="g1", name=f"pt{h}{j}")
                    nc.tensor.transpose(pt[:], tsb[:, j * 128:(j + 1) * 128],
                                        ident[:])
                    nc.vector.tensor_scalar_mul(
                        obs[j][:, h * 128:(h + 1) * 128], pt[:], osc[:, j:j + 1])
            for j in range(4):
                nc.sync.dma_start(d_out[j * 128:(j + 1) * 128, :], obs[j][:])
    nc.compile()
    _CACHED[key] = nc
    return nc


def _run(inputs, trace=False):
    from concourse.bass_utils import run_bass_kernel_spmd

    prep = _prep(**inputs)
    nc = _build_nc(prep["uniform"])
    shared = {k: prep[k] for k in ("lpack", "mv", "inv_sp")}
    in_maps = []
    for c in range(NCORES):
        sl = slice(c * BLOC, (c + 1) * BLOC)
        in_maps.append(dict(r1=np.ascontiguousarray(prep["r1"][:, sl]),
                            r2=np.ascontiguousarray(prep["r2"][:, sl]),
                            vrow=np.ascontiguousarray(prep["vrow"][:, sl]),
                            osc=np.ascontiguousarray(prep["osc"][c]),
                            **shared))
    res = run_bass_kernel_spmd(nc, in_maps, list(range(NCORES)), trace=trace)
    out = np.concatenate([res.results[c]["out"] for c in range(NCORES)], 0)
    return out.astype(np.float32), res


def kernel(**inputs):
    out, _ = _run(inputs, trace=False)
    return out


def _install_ntff_hook():
    """The agent image's antenv lacks axon_hooks; recreate it so trace=True
    can capture NTFF profiles via libaxon_pjrt.so (same mechanism as
    trn_agent_boot.trn_boot)."""
    import types

    try:
        from antenv.axon_hooks import get_axon_ntff_profile_hook  # noqa: F401
        return
    except ImportError:
        pass
    import contextlib
    import ctypes

    so_path = "/opt/axon/libaxon_pjrt.so"
    lib = ctypes.CDLL(so_path)
    lib.axon_start_nrt_profile.argtypes = [ctypes.POINTER(ctypes.c_int64),
                                           ctypes.c_size_t]
    lib.axon_start_nrt_profile.restype = ctypes.c_int64
    lib.axon_stop_nrt_profile.argtypes = [ctypes.c_char_p]
    lib.axon_stop_nrt_profile.restype = ctypes.c_int64

    @contextlib.contextmanager
    def _hook(output_dir, device_ids):
        import jax

        jax.devices()
        if device_ids:
            ids = (ctypes.c_int64 * len(device_ids))(*device_ids)
            rc = lib.axon_start_nrt_profile(ids, len(device_ids))
        else:
            rc = lib.axon_start_nrt_profile(None, 0)
        if rc != 0:
            raise RuntimeError(f"axon_start_nrt_profile rc={rc}")
        try:
            yield
        finally:
            n = lib.axon_stop_nrt_profile(str(output_dir).encode())
            if n < 0:
                raise RuntimeError(f"axon_stop_nrt_profile rc={n}")
            if n == 0:
                print("WARNING: NTFF capture wrote nothing (raced the execute)")

    mod = types.ModuleType("antenv.axon_hooks")
    mod.get_axon_ntff_profile_hook = lambda: _hook
    mod.set_axon_ntff_profile_hook = lambda h: None
    sys.modules["antenv.axon_hooks"] = mod
    import antenv

    antenv.axon_hooks = mod


def run_traced(inputs):
    _install_ntff_hook()
    return _run(inputs, trace=True)
